# revision 1
# baseline (speedup 1.0000x reference)
"""Bass/Trainium2 kernel for nn_NeuroBiMambaBlock.

Sharding: 8 cores = 4 samples x 2 directions (fwd/bwd mamba). Every core
runs an identical SPMD program on its own data: bwd cores receive the
time-flipped sample and the b_* weight set, so their mamba scan is
forward-in-layout. The outer (shared) stage is replicated per pair; its
causal conv becomes anti-causal on flipped cores, handled by a 7-tap
"wide" conv whose taps the host builds per direction. Each core returns
a partial output (its direction's contribution through the final
projection); the host sums the pair, un-flips the bwd part, and adds the
residual.

Layout on device: [feature -> partitions, time -> free]. The selective
scan runs as one tensor_tensor_scan per 128-channel block, with the 16
states per channel packed as 16 segments of (1 boundary col + T token
cols) along the free dim; the boundary column injects the carried state
(a[boundary]=0 resets, so state[boundary] = dBu[boundary] = carry).
"""

import numpy as np

B, L, DM = 4, 4096, 256
DH = 512
N = 16
KC = 4
R = 32
EPS = 1e-5
T = 256                  # tokens per pipeline tile
NT = L // T              # tiles
SEG = T + 1              # scan segment length (boundary col + T tokens)
NDB = DH // 128          # 4 channel blocks
NTB = T // 128           # token blocks per tile

_CACHE = {}


def build_program(Lx=L):
    import concourse.bass as bass
    import concourse.bacc as bacc
    import concourse.tile as tile
    import concourse.mybir as mybir
    from contextlib import ExitStack

    f32 = mybir.dt.float32
    bf16 = mybir.dt.bfloat16
    AF = mybir.ActivationFunctionType
    OP = mybir.AluOpType
    AX = mybir.AxisListType

    from concourse import library_config
    nt = Lx // T
    nc = bacc.Bacc("TRN2", target_bir_lowering=False, debug=False)

    x_in = nc.declare_dram_parameter("x_in", [Lx, DM], bf16, isOutput=False)
    w_in_T = nc.declare_dram_parameter("w_in_T", [DM, 2 * DH], bf16, isOutput=False)
    gate_bias = nc.declare_dram_parameter("gate_bias", [DH, 1], f32, isOutput=False)
    w7d = nc.declare_dram_parameter("w7", [DH, 7], f32, isOutput=False)
    conv_bd = nc.declare_dram_parameter("conv_b", [DH, 1], f32, isOutput=False)
    m_in_T = nc.declare_dram_parameter("m_in_T", [DH, 2 * DH], bf16, isOutput=False)
    m_conv_wd = nc.declare_dram_parameter("m_conv_w", [DH, KC], f32, isOutput=False)
    m_conv_bd = nc.declare_dram_parameter("m_conv_b", [DH, 1], f32, isOutput=False)
    m_xproj_T = nc.declare_dram_parameter("m_xproj_T", [DH, R + 2 * N], bf16, isOutput=False)
    m_dt_wT = nc.declare_dram_parameter("m_dt_wT", [R, DH], bf16, isOutput=False)
    m_dt_bd = nc.declare_dram_parameter("m_dt_b", [DH, 1], f32, isOutput=False)
    m_out_T2 = nc.declare_dram_parameter("m_out_T2", [DH, DH], bf16, isOutput=False)
    m_Dd = nc.declare_dram_parameter("m_D", [DH, 1], f32, isOutput=False)
    w_out_sl_T = nc.declare_dram_parameter("w_out_sl_T", [DH, DM], bf16, isOutput=False)
    part = nc.declare_dram_parameter("part", [Lx, DM], bf16, isOutput=True)

    with tile.TileContext(nc) as tc, ExitStack() as ctx:
        wpool = ctx.enter_context(tc.tile_pool(name="weights", bufs=1))
        psum = ctx.enter_context(tc.tile_pool(name="psum", bufs=2, space="PSUM"))
        pp1 = ctx.enter_context(tc.tile_pool(name="pipe1", bufs=1))
        pp2 = ctx.enter_context(tc.tile_pool(name="pipe2", bufs=2))
        cinp = ctx.enter_context(tc.tile_pool(name="cin", bufs=3))
        xinp = ctx.enter_context(tc.tile_pool(name="xin", bufs=2))
        spool = ctx.enter_context(tc.tile_pool(name="scan", bufs=2))
        sp1 = ctx.enter_context(tc.tile_pool(name="scan1", bufs=1))
        stp = ctx.enter_context(tc.tile_pool(name="state", bufs=2))
        smalls = ctx.enter_context(tc.tile_pool(name="smalls", bufs=2))

        # ---- weights to SBUF ----
        winT = []
        for kb in range(DM // 128):
            t = wpool.tile([128, 2 * DH], bf16, tag=f"winT{kb}", name=f"winT{kb}")
            nc.sync.dma_start(t[:], w_in_T[kb * 128:(kb + 1) * 128, :])
            winT.append(t)
        minT = []
        for kb in range(NDB):
            t = wpool.tile([128, 2 * DH], bf16, tag=f"minT{kb}", name=f"minT{kb}")
            nc.sync.dma_start(t[:], m_in_T[kb * 128:(kb + 1) * 128, :])
            minT.append(t)
        mxpT = []
        for kb in range(NDB):
            t = wpool.tile([128, R + 2 * N], bf16, tag=f"mxpT{kb}", name=f"mxpT{kb}")
            nc.sync.dma_start(t[:], m_xproj_T[kb * 128:(kb + 1) * 128, :])
            mxpT.append(t)
        mdtT = wpool.tile([R, DH], bf16)
        nc.sync.dma_start(mdtT[:], m_dt_wT[:])
        moT2 = []
        for kb in range(DH // 128):
            t = wpool.tile([128, DH], bf16, tag=f"moT2_{kb}", name=f"moT2_{kb}")
            nc.sync.dma_start(t[:], m_out_T2[kb * 128:(kb + 1) * 128, :])
            moT2.append(t)
        woT = []
        for kb in range(NDB):
            t = wpool.tile([128, DM], bf16, tag=f"woT{kb}", name=f"woT{kb}")
            nc.sync.dma_start(t[:], w_out_sl_T[kb * 128:(kb + 1) * 128, :])
            woT.append(t)

        _cv = [0]
        def colvec(dram):
            out = []
            for db in range(NDB):
                _cv[0] += 1
                t = wpool.tile([128, 1], f32, tag=f"cv{_cv[0]}", name=f"cv{_cv[0]}")
                nc.sync.dma_start(t[:], dram[db * 128:(db + 1) * 128, :])
                out.append(t)
            return out

        mD = colvec(m_Dd)
        gbias = colvec(gate_bias)
        cbias = colvec(conv_bd)
        mcbias = colvec(m_conv_bd)
        mdtb = colvec(m_dt_bd)
        w7c, mcw = [], []
        for db in range(NDB):
            t = wpool.tile([128, 7], f32, tag=f"w7c{db}", name=f"w7c{db}")
            nc.sync.dma_start(t[:], w7d[db * 128:(db + 1) * 128, :])
            w7c.append(t)
            t2 = wpool.tile([128, KC], f32, tag=f"mcw{db}", name=f"mcw{db}")
            nc.sync.dma_start(t2[:], m_conv_wd[db * 128:(db + 1) * 128, :])
            mcw.append(t2)

        # identity for PE transposes
        idf = wpool.tile([128, 128], f32)
        pidx = wpool.tile([128, 1], f32)
        nc.gpsimd.iota(idf[:], [[1, 128]], channel_multiplier=0,
                       allow_small_or_imprecise_dtypes=True)
        nc.gpsimd.iota(pidx[:], [[0, 1]], channel_multiplier=1,
                       allow_small_or_imprecise_dtypes=True)
        ident = wpool.tile([128, 128], bf16)
        nc.vector.tensor_scalar(ident[:], idf[:], pidx[:], None, OP.is_equal)
        # selector [16, 16*128]: sel[k, n*128+m] = (k == n), for PE row-broadcast
        self_f = wpool.tile([16, N * 128], f32)
        nc.gpsimd.iota(self_f[:], [[1, N], [0, 128]], channel_multiplier=0,
                       allow_small_or_imprecise_dtypes=True)
        pidx16 = wpool.tile([16, 1], f32)
        nc.gpsimd.iota(pidx16[:], [[0, 1]], channel_multiplier=1,
                       allow_small_or_imprecise_dtypes=True)
        sel = wpool.tile([16, N * 128], bf16)
        nc.vector.tensor_scalar(sel[:], self_f[:], pidx16[:], None, OP.is_equal)

        nblk = Lx // 128
        xfull = wpool.tile([128, nblk * DM], bf16)
        nc.sync.dma_start(
            xfull[:].rearrange("p (a d) -> p a d", d=DM),
            x_in[:].rearrange("(a p) d -> p a d", p=128))
        ofull = wpool.tile([128, nblk * DM], bf16)

        cin_prev = [None] * NDB
        sg_prev = [None] * NDB
        xin_prev = [None] * NDB
        st_prev = [None] * NDB     # carried scan states [128,16] per dblk

        def seg3(ap):
            return ap[:].rearrange("p (n c) -> p n c", c=SEG)

        def run_tail(j, cin_j, sg_j):
            """All stages of tile j that need the outer-conv future halo."""
            # ---- outer wide conv (7 taps) + silu ----
            actT = [pp1.tile([128, T], bf16, tag=f"actT{db}", name=f"actT{db}") for db in range(NDB)]
            for db in range(NDB):
                acc = pp2.tile([128, T], f32, tag="cacc")
                nc.vector.tensor_scalar(acc[:], cin_j[db][:, 3:3 + T],
                                        w7c[db][:, 0:1], None, OP.mult)
                for k in range(1, 7):
                    acc2 = pp2.tile([128, T], f32, tag="cacc")
                    nc.vector.scalar_tensor_tensor(
                        out=acc2[:], in0=cin_j[db][:, 3 + k:3 + k + T],
                        scalar=w7c[db][:, k:k + 1], in1=acc[:],
                        op0=OP.mult, op1=OP.add)
                    acc = acc2
                a_sig = pp2.tile([128, T], f32, tag="a_sig", name="a_sig")
                nc.scalar.activation(a_sig[:], acc[:], AF.Sigmoid, bias=cbias[db][:])
                nc.vector.scalar_tensor_tensor(
                    out=actT[db][:], in0=acc[:], scalar=cbias[db][:],
                    in1=a_sig[:], op0=OP.add, op1=OP.mult)

            # ---- inner in-proj ----
            xin = [xinp.tile([128, T + 3], f32, tag=f"xin{db}", name=f"xin{db}") for db in range(NDB)]
            szT = [pp1.tile([128, T], bf16, tag=f"szT{db}", name=f"szT{db}") for db in range(NDB)]
            for mb in range(2 * DH // 128):
                pt = psum.tile([128, T], f32, tag="mm")
                for kb in range(NDB):
                    nc.tensor.matmul(pt[:], minT[kb][:, mb * 128:(mb + 1) * 128],
                                     actT[kb][:], start=(kb == 0), stop=(kb == NDB - 1))
                if mb < NDB:
                    nc.vector.tensor_copy(xin[mb][:, 3:3 + T], pt[:])
                else:
                    z_sig = pp2.tile([128, T], f32, tag="z_sig", name="z_sig")
                    nc.scalar.activation(z_sig[:], pt[:], AF.Sigmoid)
                    nc.vector.tensor_tensor(out=szT[mb - NDB][:], in0=pt[:],
                                            in1=z_sig[:], op=OP.mult)
            for db in range(NDB):
                if j == 0:
                    nc.gpsimd.memset(xin[db][:, 0:3], 0.0)
                else:
                    nc.vector.tensor_copy(xin[db][:, 0:3], xin_prev[db][:, T:T + 3])
                xin_prev[db] = xin[db]

            # ---- inner causal conv (4 taps) + silu ----
            uT = [pp1.tile([128, T], bf16, tag=f"uT{db}", name=f"uT{db}") for db in range(NDB)]
            for db in range(NDB):
                acc = pp2.tile([128, T], f32, tag="macc")
                nc.vector.tensor_scalar(acc[:], xin[db][:, 0:T],
                                        mcw[db][:, 0:1], None, OP.mult)
                for k in range(1, KC):
                    acc2 = pp2.tile([128, T], f32, tag="macc")
                    nc.vector.scalar_tensor_tensor(
                        out=acc2[:], in0=xin[db][:, k:k + T],
                        scalar=mcw[db][:, k:k + 1], in1=acc[:],
                        op0=OP.mult, op1=OP.add)
                    acc = acc2
                u_sig = pp2.tile([128, T], f32, tag="u_sig", name="u_sig")
                nc.scalar.activation(u_sig[:], acc[:], AF.Sigmoid, bias=mcbias[db][:])
                nc.vector.scalar_tensor_tensor(
                    out=uT[db][:], in0=acc[:], scalar=mcbias[db][:],
                    in1=u_sig[:], op0=OP.add, op1=OP.mult)

            # ---- xproj (split: dt-rank rows / B rows / C rows) ----
            pxd = psum.tile([R, T], f32, tag="mm")
            pxb = psum.tile([N, T], f32, tag="mm")
            pxc = psum.tile([N, T], f32, tag="mm")
            for kb in range(NDB):
                nc.tensor.matmul(pxd[:], mxpT[kb][:, 0:R], uT[kb][:],
                                 start=(kb == 0), stop=(kb == NDB - 1))
            for kb in range(NDB):
                nc.tensor.matmul(pxb[:], mxpT[kb][:, R:R + N], uT[kb][:],
                                 start=(kb == 0), stop=(kb == NDB - 1))
            for kb in range(NDB):
                nc.tensor.matmul(pxc[:], mxpT[kb][:, R + N:], uT[kb][:],
                                 start=(kb == 0), stop=(kb == NDB - 1))
            xdbl = pp1.tile([R, T], bf16, tag="xdbl")
            nc.vector.tensor_copy(xdbl[:], pxd[:])
            xdB = pp1.tile([N, T], bf16, tag="xdB")
            nc.vector.tensor_copy(xdB[:], pxb[:])
            xdC = pp1.tile([N, T], bf16, tag="xdC")
            nc.vector.tensor_copy(xdC[:], pxc[:])

            # ---- dt / r ----
            mdtT_t = [pp1.tile([128, T], f32, tag=f"mdtT_t{db}", name=f"mdtT_t{db}") for db in range(NDB)]
            rT = [pp1.tile([128, T], bf16, tag=f"rT{db}", name=f"rT{db}") for db in range(NDB)]
            for db in range(NDB):
                pt = psum.tile([128, T], f32, tag="mm")
                nc.tensor.matmul(pt[:], mdtT[:, db * 128:(db + 1) * 128],
                                 xdbl[:], start=True, stop=True)
                rf = pp2.tile([128, T], f32, tag="rf", name="rf")
                nc.scalar.activation(rf[:], pt[:], AF.Sigmoid, scale=-1.0,
                                     bias=mdtb[db][:])
                nc.scalar.activation(mdtT_t[db][:], rf[:], AF.Ln)
                nc.vector.tensor_copy(rT[db][:], rf[:])

            dtuT = [pp1.tile([128, T], bf16, tag=f"dtuT{db}", name=f"dtuT{db}") for db in range(NDB)]
            for db in range(NDB):
                nc.vector.tensor_tensor(out=dtuT[db][:], in0=mdtT_t[db][:],
                                        in1=uT[db][:], op=OP.mult)

            # B broadcast (PE selector) consumed straight from PSUM by dBu;
            # C broadcast materialized into crep segments.
            crep = sp1.tile([128, N * SEG], bf16, tag="crep")
            nc.vector.memset(seg3(crep)[:, :, 0:1], 0.0)
            dbus = [sp1.tile([128, N * SEG], bf16, tag=f"dbu{db}", name=f"dbu{db}")
                    for db in range(NDB)]
            for db in range(NDB):
                if j == 0:
                    nc.vector.memset(seg3(dbus[db])[:, :, 0:1], 0.0)
                else:
                    nc.vector.tensor_copy(seg3(dbus[db])[:, :, 0:1],
                                          st_prev[db][:].rearrange("p (n o) -> p n o", o=1))
            for n in range(N):
                pb = psum.tile([128, T], f32, tag="bc")
                nc.tensor.matmul(pb[:], sel[:, n * 128:(n + 1) * 128], xdB[:],
                                 start=True, stop=True)
                for db in range(NDB):
                    nc.vector.tensor_tensor(
                        out=dbus[db][:, n * SEG + 1:(n + 1) * SEG],
                        in0=dtuT[db][:], in1=pb[:], op=OP.mult)
                pc = psum.tile([128, T], f32, tag="bc")
                nc.tensor.matmul(pc[:], sel[:, n * 128:(n + 1) * 128], xdC[:],
                                 start=True, stop=True)
                nc.vector.tensor_copy(crep[:, n * SEG + 1:(n + 1) * SEG], pc[:])

            # ---- scan per channel block ----
            yT = [pp1.tile([128, T], bf16, tag=f"yT{db}", name=f"yT{db}") for db in range(NDB)]
            for db in range(NDB):
                a_t = spool.tile([128, N * SEG], bf16, tag="a")
                nc.gpsimd.memset(seg3(a_t)[:, :, 0:1], 0.0)
                nc.vector.tensor_copy(a_t[:, 1:SEG], rT[db][:])
                for n in range(1, N):
                    nc.vector.tensor_tensor(
                        out=a_t[:, n * SEG + 1:(n + 1) * SEG],
                        in0=a_t[:, (n - 1) * SEG + 1:n * SEG],
                        in1=rT[db][:], op=OP.mult)
                h_t = spool.tile([128, N * SEG], bf16, tag="h")
                nc.vector.tensor_tensor_scan(h_t[:], a_t[:], dbus[db][:], 0.0,
                                             OP.mult, OP.add)
                st = stp.tile([128, N], bf16, tag=f"st{db}")
                nc.vector.tensor_copy(st[:].rearrange("p (n o) -> p n o", o=1),
                                      seg3(h_t)[:, :, SEG - 1:SEG])
                st_prev[db] = st
                hc = spool.tile([128, N * SEG], bf16, tag="a")
                nc.vector.tensor_tensor(out=hc[:], in0=h_t[:], in1=crep[:], op=OP.mult)
                h3 = seg3(hc)
                nn = N
                while nn > 2:
                    nn //= 2
                    nc.vector.tensor_tensor(
                        out=h3[:, 0:nn, 1:SEG], in0=h3[:, 0:nn, 1:SEG],
                        in1=h3[:, nn:2 * nn, 1:SEG], op=OP.add)
                nc.vector.tensor_tensor(out=yT[db][:], in0=h3[:, 0:1, 1:SEG],
                                        in1=h3[:, 1:2, 1:SEG], op=OP.add)
                nc.vector.scalar_tensor_tensor(
                    out=yT[db][:], in0=uT[db][:], scalar=mD[db][:],
                    in1=yT[db][:], op0=OP.mult, op1=OP.add)

            # ---- gating + out-proj (+ D-term) ----
            g1 = [pp1.tile([128, T], bf16, tag=f"g1{db}", name=f"g1{db}") for db in range(NDB)]
            for db in range(NDB):
                nc.vector.tensor_tensor(out=g1[db][:], in0=yT[db][:],
                                        in1=szT[db][:], op=OP.mult)
            moT = [pp1.tile([128, T], bf16, tag=f"moT{db}", name=f"moT{db}") for db in range(NDB)]
            for mb in range(NDB):
                pt = psum.tile([128, T], f32, tag="mm")
                for kb in range(NDB):
                    nc.tensor.matmul(pt[:], moT2[kb][:, mb * 128:(mb + 1) * 128],
                                     g1[kb][:], start=(kb == 0), stop=(kb == NDB - 1))
                nc.vector.tensor_tensor(out=moT[mb][:], in0=pt[:],
                                        in1=sg_j[mb][:], op=OP.mult)

            # ---- final projection + transpose + DMA out ----
            for mb in range(DM // 128):
                pt = psum.tile([128, T], f32, tag="mm")
                for kb in range(NDB):
                    nc.tensor.matmul(pt[:], woT[kb][:, mb * 128:(mb + 1) * 128],
                                     moT[kb][:], start=(kb == 0), stop=(kb == NDB - 1))
                ot = pp1.tile([128, T], bf16, tag="ot")
                nc.vector.tensor_copy(ot[:], pt[:])
                for tb in range(NTB):
                    pt2 = psum.tile([128, 128], bf16, tag="tr")
                    nc.tensor.transpose(pt2[:], ot[:, tb * 128:(tb + 1) * 128], ident[:])
                    blk = j * NTB + tb
                    nc.vector.tensor_copy(
                        ofull[:, blk * DM + mb * 128: blk * DM + (mb + 1) * 128],
                        pt2[:])

        # ================= main loop =================
        for i in range(nt):
            # LN + transpose
            hT = [pp2.tile([128, T], bf16, tag=f"hT{db}", name=f"hT{db}") for db in range(DM // 128)]
            for tb in range(NTB):
                blk = i * NTB + tb
                xt = xfull[:, blk * DM:(blk + 1) * DM]
                s1 = smalls.tile([128, 1], f32, tag="s1")
                nc.vector.tensor_reduce(s1[:], xt, axis=AX.X, op=OP.add)
                sq = pp2.tile([128, DM], f32, tag="sq", bufs=4)
                nc.vector.tensor_tensor(out=sq[:], in0=xt, in1=xt, op=OP.mult)
                s2 = smalls.tile([128, 1], f32, tag="s2")
                nc.vector.tensor_reduce(s2[:], sq[:], axis=AX.X, op=OP.add)
                m = smalls.tile([128, 1], f32, tag="m")
                nc.scalar.mul(m[:], s1[:], 1.0 / DM)
                v = smalls.tile([128, 1], f32, tag="v")
                nc.vector.tensor_scalar(v[:], m[:], m[:], None, OP.mult)
                v2 = smalls.tile([128, 1], f32, tag="v2")
                nc.vector.tensor_scalar(v2[:], s2[:], 1.0 / DM, None, OP.mult)
                v3 = smalls.tile([128, 1], f32, tag="v3")
                nc.vector.tensor_tensor(out=v3[:], in0=v2[:], in1=v[:], op=OP.subtract)
                v4 = smalls.tile([128, 1], f32, tag="v4")
                nc.vector.tensor_scalar(v4[:], v3[:], EPS, None, OP.add)
                rv = smalls.tile([128, 1], f32, tag="rv")
                nc.vector.reciprocal(rv[:], v4[:])
                rstd = smalls.tile([128, 1], f32, tag="rstd")
                nc.scalar.activation(rstd[:], rv[:], AF.Sqrt)
                xln = pp2.tile([128, DM], bf16, tag="xln")
                nc.vector.tensor_scalar(xln[:], xt, m[:], rstd[:],
                                        OP.subtract, OP.mult)
                for db in range(DM // 128):
                    ptr = psum.tile([128, 128], bf16, tag="tr")
                    nc.tensor.transpose(ptr[:], xln[:, db * 128:(db + 1) * 128], ident[:])
                    nc.vector.tensor_copy(hT[db][:, tb * 128:(tb + 1) * 128], ptr[:])

            # outer in-proj -> cin (conv part) + silu(gate)
            cin = [cinp.tile([128, T + 9], f32, tag=f"cin{db}", name=f"cin{db}") for db in range(NDB)]
            sgT = [pp2.tile([128, T], bf16, tag=f"sgT{db}", name=f"sgT{db}") for db in range(NDB)]
            for mb in range(2 * DH // 128):
                pt = psum.tile([128, T], f32, tag="mm")
                for kb in range(DM // 128):
                    nc.tensor.matmul(pt[:], winT[kb][:, mb * 128:(mb + 1) * 128],
                                     hT[kb][:], start=(kb == 0), stop=(kb == DM // 128 - 1))
                if mb < NDB:
                    nc.vector.tensor_copy(cin[mb][:, 6:6 + T], pt[:])
                else:
                    db = mb - NDB
                    sg_sig = pp2.tile([128, T], f32, tag="sg_sig", name="sg_sig")
                    nc.scalar.activation(sg_sig[:], pt[:], AF.Sigmoid, bias=gbias[db][:])
                    nc.vector.scalar_tensor_tensor(
                        out=sgT[db][:], in0=pt[:], scalar=gbias[db][:],
                        in1=sg_sig[:], op0=OP.add, op1=OP.mult)

            for db in range(NDB):
                if i == 0:
                    nc.gpsimd.memset(cin[db][:, 0:6], 0.0)
                else:
                    nc.vector.tensor_copy(cin[db][:, 0:6], cin_prev[db][:, T:T + 6])
                    nc.vector.tensor_copy(cin_prev[db][:, T + 6:T + 9], cin[db][:, 6:9])

            if i > 0:
                run_tail(i - 1, cin_prev, sg_prev)
            cin_prev = cin
            sg_prev = sgT

        for db in range(NDB):
            nc.gpsimd.memset(cin_prev[db][:, T + 6:T + 9], 0.0)
        run_tail(nt - 1, cin_prev, sg_prev)
        nc.sync.dma_start(
            part[:].rearrange("(a p) d -> p a d", p=128),
            ofull[:].rearrange("p (a d) -> p a d", d=DM))

    nc.compile()
    return nc


def host_prepare(inputs, Lx=L):
    import ml_dtypes
    f32 = np.float32
    bf = ml_dtypes.bfloat16
    x = np.asarray(inputs["x"], f32)
    ln_g = np.asarray(inputs["ln_g"], f32)
    ln_b = np.asarray(inputs["ln_b"], f32)
    in_w = np.asarray(inputs["in_w"], f32)
    conv_w = np.asarray(inputs["conv_w"], f32)
    conv_b = np.asarray(inputs["conv_b"], f32)
    out_w = np.asarray(inputs["out_w"], f32)

    in_w_eff = in_w * ln_g[None, :]
    bias_vec = in_w @ ln_b

    core_maps, meta = [], []
    for b in range(x.shape[0]):
        for d, p in enumerate(("f", "b")):
            m_in_w = np.asarray(inputs[p + "_in_w"], f32)
            m_conv_w = np.asarray(inputs[p + "_conv_w"], f32)
            m_conv_b = np.asarray(inputs[p + "_conv_b"], f32)
            m_xproj = np.asarray(inputs[p + "_xproj_w"], f32)
            m_dt_w = np.asarray(inputs[p + "_dt_w"], f32)
            m_dt_b = np.asarray(inputs[p + "_dt_b"], f32)
            m_D = np.asarray(inputs[p + "_D"], f32)
            m_out_w = np.asarray(inputs[p + "_out_w"], f32)

            xc = x[b] if d == 0 else x[b, ::-1]
            w7 = np.zeros((DH, 7), f32)
            if d == 0:
                w7[:, 0:4] = conv_w
            else:
                w7[:, 3:7] = conv_w[:, ::-1]
            cb_eff = conv_b + bias_vec[:DH] * conv_w.sum(axis=1)
            mo2 = -m_out_w.T

            core_maps.append({
                "x_in": np.ascontiguousarray(xc).astype(bf),
                "w_in_T": np.ascontiguousarray(in_w_eff.T).astype(bf),
                "gate_bias": np.ascontiguousarray(bias_vec[DH:, None], f32),
                "w7": w7,
                "conv_b": np.ascontiguousarray(cb_eff[:, None], f32),
                "m_in_T": np.ascontiguousarray(m_in_w.T).astype(bf),
                "m_conv_w": np.ascontiguousarray(m_conv_w, f32),
                "m_conv_b": np.ascontiguousarray(m_conv_b[:, None], f32),
                "m_xproj_T": np.ascontiguousarray(m_xproj.T).astype(bf),
                "m_dt_wT": np.ascontiguousarray(m_dt_w.T).astype(bf),
                "m_dt_b": np.ascontiguousarray(-m_dt_b[:, None], f32),
                "m_out_T2": np.ascontiguousarray(mo2).astype(bf),
                "m_D": np.ascontiguousarray(-m_D[:, None], f32),
                "w_out_sl_T": np.ascontiguousarray(
                    out_w[:, d * DH:(d + 1) * DH].T).astype(bf),
            })
            meta.append((b, d))
    return core_maps, meta


def kernel(**inputs) -> np.ndarray:
    from concourse.bass_utils import run_bass_kernel_spmd

    if "nc" not in _CACHE:
        _CACHE["nc"] = build_program()
    nc = _CACHE["nc"]

    core_maps, meta = host_prepare(inputs)
    res = run_bass_kernel_spmd(nc, core_maps, list(range(len(core_maps))))
    x = np.asarray(inputs["x"], np.float32)
    out = np.array(x, np.float32, copy=True)
    for i, (b, d) in enumerate(meta):
        p = np.asarray(res.results[i]["part"], np.float32)
        out[b] += p if d == 0 else p[::-1]
    return out



# revision 3
# speedup vs baseline: 2222.0587x; 2222.0587x over previous
"""Bass/Trainium2 kernel for nn_NeuroBiMambaBlock — engine-balanced rewrite.

Sharding: 8 cores = 4 samples x 2 directions (identical SPMD program; bwd
cores get time-flipped input + b_* weights). Within a core:
  PE   : all matmuls, depthwise convs as diag-matmul PSUM accumulation,
         B/C partition broadcasts, n-sum tree as identity-matmul PSUM
         accumulation (+ diag(-D) term), bias adds as rank-1 matmuls.
  DVE  : selective-scan (one mega tensor_tensor_scan over 4 channel
         blocks), big 2x-mode tensor_tensor streams (decay powers via
         doubling with broadcast APs, dt*u*B, h*C).
  Act  : sigmoids + Ln (2 table loads/tile), PSUM->SBUF escapes.
  Pool : overflow tensor_tensor work, boundary/carry chores.
LN stats precomputed in a prologue (Sqrt table loaded once up front).
"""

import numpy as np

B, L, DM = 4, 4096, 256
DH = 512
N = 16
KC = 4
R = 32
EPS = 1e-5
T = 256
NT = L // T              # 16 tiles
SEG = T + 1
ND = DH // 128           # 4 channel blocks
NTB = T // 128           # 2 token blocks / tile
CW = 7                   # outer wide-conv taps

_CACHE = {}
DBG = False


def build_program(Lx=L):
    import concourse.bass as bass
    import concourse.bacc as bacc
    import concourse.tile as tile
    import concourse.mybir as mybir
    from contextlib import ExitStack

    f32 = mybir.dt.float32
    bf16 = mybir.dt.bfloat16
    AF = mybir.ActivationFunctionType
    OP = mybir.AluOpType
    AX = mybir.AxisListType

    nt = Lx // T
    nblk = Lx // 128
    nc = bacc.Bacc("TRN2", target_bir_lowering=False, debug=False)

    # ---- DRAM ----
    x_in = nc.declare_dram_parameter("x_in", [Lx, DM], bf16, isOutput=False)
    w_in_T = nc.declare_dram_parameter("w_in_T", [DM, 2 * DH], bf16, isOutput=False)
    m_in_T = nc.declare_dram_parameter("m_in_T", [DH, 2 * DH], bf16, isOutput=False)
    m_xproj_T = nc.declare_dram_parameter("m_xproj_T", [DH, R + 2 * N], bf16, isOutput=False)
    m_dt_wT = nc.declare_dram_parameter("m_dt_wT", [R, DH], bf16, isOutput=False)
    m_out_T2 = nc.declare_dram_parameter("m_out_T2", [DH, DH], bf16, isOutput=False)
    w_out_sl_T = nc.declare_dram_parameter("w_out_sl_T", [DH, DM], bf16, isOutput=False)
    diags_d = nc.declare_dram_parameter("diags", [128, 49 * 128], bf16, isOutput=False)
    bias_d = nc.declare_dram_parameter("bias_rows", [1, 4 * DH], bf16, isOutput=False)
    sel_d = nc.declare_dram_parameter("sel", [48, N * 128], bf16, isOutput=False)
    part = nc.declare_dram_parameter("part", [Lx, DM], bf16, isOutput=True)
    dbg = {}
    if DBG:
        for nm, shp in (("hT", [128, 2 * T]), ("actT", [128, ND * T]),
                        ("uT", [128, ND * T]), ("sgT", [128, ND * T]),
                        ("szT", [128, ND * T]), ("rfq", [128, ND * T]),
                        ("mdt", [128, ND * T]), ("dtu", [128, ND * T]),
                        ("brep", [128, N * T]), ("crep", [128, N * T]),
                        ("amega", [128, ND * N * SEG]),
                        ("hsc", [128, ND * N * SEG]),
                        ("yq", [128, ND * T]), ("g1", [128, ND * T]),
                        ("moq", [128, ND * T]), ("xbc", [48, T])):
            dbg[nm] = nc.declare_dram_parameter("dbg_" + nm, shp, bf16, isOutput=True)

    TC9 = T + 9
    TC3 = T + 3

    with tile.TileContext(nc) as tc, ExitStack() as ctx:
        wp = ctx.enter_context(tc.tile_pool(name="w", bufs=1))
        ps = ctx.enter_context(tc.tile_pool(name="ps", bufs=3, space="PSUM"))
        psb = ctx.enter_context(tc.tile_pool(name="psb", bufs=2, space="PSUM"))
        pst = ctx.enter_context(tc.tile_pool(name="pst", bufs=2, space="PSUM"))
        sm = ctx.enter_context(tc.tile_pool(name="sm", bufs=2))
        big = ctx.enter_context(tc.tile_pool(name="big", bufs=1))
        hal = ctx.enter_context(tc.tile_pool(name="hal", bufs=2))

        # ---- weights ----
        winT = []
        for kb in range(DM // 128):
            t = wp.tile([128, 2 * DH], bf16, tag=f"winT{kb}")
            nc.sync.dma_start(t[:], w_in_T[kb * 128:(kb + 1) * 128, :])
            winT.append(t)
        minT = []
        for kb in range(ND):
            t = wp.tile([128, 2 * DH], bf16, tag=f"minT{kb}")
            nc.sync.dma_start(t[:], m_in_T[kb * 128:(kb + 1) * 128, :])
            minT.append(t)
        mxpT = []
        for kb in range(ND):
            t = wp.tile([128, R + 2 * N], bf16, tag=f"mxpT{kb}")
            nc.sync.dma_start(t[:], m_xproj_T[kb * 128:(kb + 1) * 128, :])
            mxpT.append(t)
        mdtT = wp.tile([R, DH], bf16, tag="mdtT")
        nc.sync.dma_start(mdtT[:], m_dt_wT[:])
        moT2 = []
        for kb in range(ND):
            t = wp.tile([128, DH], bf16, tag=f"moT2_{kb}")
            nc.sync.dma_start(t[:], m_out_T2[kb * 128:(kb + 1) * 128, :])
            moT2.append(t)
        woT = []
        for kb in range(ND):
            t = wp.tile([128, DM], bf16, tag=f"woT{kb}")
            nc.sync.dma_start(t[:], w_out_sl_T[kb * 128:(kb + 1) * 128, :])
            woT.append(t)
        diags = wp.tile([128, 49 * 128], bf16, tag="diags")
        nc.sync.dma_start(diags[:], diags_d[:])

        def dg(i):
            return diags[:, i * 128:(i + 1) * 128]
        ident = dg(48)
        biasr = wp.tile([1, 4 * DH], bf16, tag="biasr")
        nc.sync.dma_start(biasr[:], bias_d[:])
        sel = wp.tile([48, N * 128], bf16, tag="sel")
        nc.sync.dma_start(sel[:], sel_d[:])
        ones_row = wp.tile([1, T], bf16, tag="ones")
        nc.vector.memset(ones_row[:], 1.0)

        # ---- prologue: LN stats for all 128-token blocks ----
        mcol = wp.tile([128, nblk], f32, tag="mcol")
        vcol = wp.tile([128, nblk], f32, tag="vcol")
        rcol = wp.tile([128, nblk], f32, tag="rcol")
        for bp in range(nblk // 2):
            xb = sm.tile([128, 2 * DM], bf16, tag="praw", bufs=2)
            nc.sync.dma_start(xb[:].rearrange("p (a d) -> p a d", d=DM),
                              x_in[bp * 256:(bp + 1) * 256, :]
                              .rearrange("(a p) d -> p a d", p=128))
            sq = sm.tile([128, 2 * DM], bf16, tag="praw", bufs=2)
            nc.gpsimd.tensor_tensor(out=sq[:], in0=xb[:], in1=xb[:], op=OP.mult)
            for o in range(2):
                blk = bp * 2 + o
                s1 = sm.tile([128, 1], f32, tag="s1")
                nc.vector.tensor_reduce(s1[:], xb[:, o * DM:(o + 1) * DM],
                                        axis=AX.X, op=OP.add)
                nc.gpsimd.tensor_scalar(mcol[:, blk:blk + 1], s1[:], 1.0 / DM,
                                        None, OP.mult)
                s2 = sm.tile([128, 1], f32, tag="s2")
                nc.vector.tensor_reduce(s2[:], sq[:, o * DM:(o + 1) * DM],
                                        axis=AX.X, op=OP.add)
                nc.gpsimd.tensor_scalar(vcol[:, blk:blk + 1], s2[:], 1.0 / DM,
                                        None, OP.mult)
        msq = wp.tile([128, nblk], f32, tag="msq")
        nc.gpsimd.tensor_tensor(out=msq[:], in0=mcol[:], in1=mcol[:], op=OP.mult)
        nc.gpsimd.tensor_tensor(out=vcol[:], in0=vcol[:], in1=msq[:], op=OP.subtract)
        nc.gpsimd.tensor_scalar(vcol[:], vcol[:], EPS, None, OP.add)
        nc.vector.reciprocal(msq[:], vcol[:])
        nc.scalar.activation(rcol[:], msq[:], AF.Sqrt)

        carry = wp.tile([128, ND * N], bf16, tag="carry")

        def r4(ap_tile):
            return ap_tile[:].rearrange("p (d n c) -> p d n c", n=N, c=SEG)

        state = {}
        xin_prev = [None]

        def stage_a(i):
            """LN + transpose + outer in-proj conv half."""
            xtile = sm.tile([128, NTB * DM], bf16, tag="xt", bufs=1)
            nc.sync.dma_start(
                xtile[:].rearrange("p (a d) -> p a d", d=DM),
                x_in[i * T:(i + 1) * T, :].rearrange("(a p) d -> p a d", p=128))
            hT = sm.tile([128, (DM // 128) * T], bf16, tag="hT")
            for tb in range(NTB):
                blk = i * NTB + tb
                xln = sm.tile([128, DM], bf16, tag="xln", bufs=1)
                nc.vector.tensor_scalar(xln[:], xtile[:, tb * DM:(tb + 1) * DM],
                                        mcol[:, blk:blk + 1], rcol[:, blk:blk + 1],
                                        OP.subtract, OP.mult)
                for fb in range(DM // 128):
                    nc.sync.dma_start(
                        hT[:, fb * T + tb * 128: fb * T + (tb + 1) * 128],
                        xln[:, fb * 128:(fb + 1) * 128], transpose=True)
            pcv = ps.tile([128, ND * T], f32, tag="big")
            for mb in range(ND):
                dst = pcv[:, mb * T:(mb + 1) * T]
                for kb in range(DM // 128):
                    nc.tensor.matmul(dst, winT[kb][:, mb * 128:(mb + 1) * 128],
                                     hT[:, kb * T:(kb + 1) * T],
                                     start=(kb == 0), stop=(kb == DM // 128 - 1))
            cin = hal.tile([128, ND * TC9], bf16, tag="cin")
            nc.scalar.copy(
                cin[:].rearrange("p (d c) -> p d c", c=TC9)[:, :, 6:6 + T],
                pcv[:].rearrange("p (d t) -> p d t", t=T))
            state[i] = {"cin": cin, "hT": hT}

        def stage_b1(j):
            st = state[j]
            cin, hT = st["cin"], st["hT"]
            c3 = cin[:].rearrange("p (d c) -> p d c", c=TC9)
            pgt = ps.tile([128, ND * T], f32, tag="big")
            for mb in range(ND):
                dst = pgt[:, mb * T:(mb + 1) * T]
                nc.tensor.matmul(dst, winT[0][:, DH + mb * 128:DH + (mb + 1) * 128],
                                 hT[:, 0:T], start=True, stop=False)
                nc.tensor.matmul(dst, winT[1][:, DH + mb * 128:DH + (mb + 1) * 128],
                                 hT[:, T:2 * T], start=False, stop=False)
                nc.tensor.matmul(dst, biasr[:, mb * 128:(mb + 1) * 128],
                                 ones_row[:], start=False, stop=True)
            sgT = sm.tile([128, ND * T], bf16, tag="sgT", bufs=5)
            nc.scalar.activation(sgT[:], pgt[:], AF.Silu)
            pca = ps.tile([128, ND * T], f32, tag="big")
            for db in range(ND):
                dst = pca[:, db * T:(db + 1) * T]
                for k in range(CW):
                    nc.tensor.matmul(dst, dg(db * 7 + k), c3[:, db, 3 + k:3 + k + T],
                                     start=(k == 0), stop=False)
                nc.tensor.matmul(dst, biasr[:, DH + db * 128:DH + (db + 1) * 128],
                                 ones_row[:], start=False, stop=True)
            actT = sm.tile([128, ND * T], bf16, tag="actT")
            nc.scalar.activation(actT[:], pca[:], AF.Silu)
            if DBG and j == 0:
                nc.sync.dma_start(dbg["hT"][:], st["hT"][:])
                nc.sync.dma_start(dbg["actT"][:], actT[:])
                nc.sync.dma_start(dbg["sgT"][:], sgT[:])
            st.update(sgT=sgT, actT=actT)

        def stage_b2(j):
            st = state[j]
            actT = st["actT"]
            px = ps.tile([128, ND * T], f32, tag="big")
            pz = ps.tile([128, ND * T], f32, tag="big")
            for mb in range(ND):
                dst = px[:, mb * T:(mb + 1) * T]
                for kb in range(ND):
                    nc.tensor.matmul(dst, minT[kb][:, mb * 128:(mb + 1) * 128],
                                     actT[:, kb * T:(kb + 1) * T],
                                     start=(kb == 0), stop=(kb == ND - 1))
            for mb in range(ND):
                dst = pz[:, mb * T:(mb + 1) * T]
                for kb in range(ND):
                    nc.tensor.matmul(dst, minT[kb][:, DH + mb * 128:DH + (mb + 1) * 128],
                                     actT[:, kb * T:(kb + 1) * T],
                                     start=(kb == 0), stop=(kb == ND - 1))
            xin = hal.tile([128, ND * TC3], bf16, tag="xin")
            x3 = xin[:].rearrange("p (d c) -> p d c", c=TC3)
            nc.scalar.copy(x3[:, :, 3:3 + T],
                           px[:].rearrange("p (d t) -> p d t", t=T))
            if j == 0:
                nc.gpsimd.memset(x3[:, :, 0:3], 0.0)
            else:
                nc.gpsimd.tensor_copy(
                    x3[:, :, 0:3],
                    xin_prev[0][:].rearrange("p (d c) -> p d c", c=TC3)[:, :, T:T + 3])
            xin_prev[0] = xin
            szT = sm.tile([128, ND * T], bf16, tag="szT", bufs=4)
            nc.scalar.activation(szT[:], pz[:], AF.Silu)
            pu = ps.tile([128, ND * T], f32, tag="big")
            for db in range(ND):
                dst = pu[:, db * T:(db + 1) * T]
                for k in range(KC):
                    nc.tensor.matmul(dst, dg(28 + db * 4 + k), x3[:, db, k:k + T],
                                     start=(k == 0), stop=False)
                nc.tensor.matmul(dst, biasr[:, 2 * DH + db * 128:2 * DH + (db + 1) * 128],
                                 ones_row[:], start=False, stop=True)
            uT = sm.tile([128, ND * T], bf16, tag="uT", bufs=4)
            nc.scalar.activation(uT[:], pu[:], AF.Silu)
            if DBG and j == 0:
                nc.sync.dma_start(dbg["uT"][:], uT[:])
                nc.sync.dma_start(dbg["szT"][:], szT[:])
            st.update(szT=szT, uT=uT)

        def stage_b3(j):
            st = state[j]
            uT = st["uT"]
            pq = psb.tile([128, 2 * T], f32, tag="bc")
            for kb in range(ND):
                nc.tensor.matmul(pq[0:R, 0:T], mxpT[kb][:, 0:R],
                                 uT[:, kb * T:(kb + 1) * T],
                                 start=(kb == 0), stop=(kb == ND - 1))
            for kb in range(ND):
                nc.tensor.matmul(pq[0:N, T:2 * T], mxpT[kb][:, R:R + N],
                                 uT[:, kb * T:(kb + 1) * T],
                                 start=(kb == 0), stop=(kb == ND - 1))
            for kb in range(ND):
                nc.tensor.matmul(pq[32:48, T:2 * T], mxpT[kb][:, R + N:],
                                 uT[:, kb * T:(kb + 1) * T],
                                 start=(kb == 0), stop=(kb == ND - 1))
            xdt = sm.tile([R, T], bf16, tag="xdt", bufs=1)
            nc.scalar.copy(xdt[:], pq[0:R, 0:T])
            xbc = sm.tile([48, T], bf16, tag="xbc", bufs=2)
            nc.scalar.copy(xbc[0:N, :], pq[0:N, T:2 * T])
            nc.scalar.copy(xbc[32:48, :], pq[32:48, T:2 * T])

            pzz = ps.tile([128, ND * T], f32, tag="big")
            for db in range(ND):
                dst = pzz[:, db * T:(db + 1) * T]
                nc.tensor.matmul(dst, mdtT[:, db * 128:(db + 1) * 128],
                                 xdt[:], start=True, stop=False)
                nc.tensor.matmul(dst, biasr[:, 3 * DH + db * 128:3 * DH + (db + 1) * 128],
                                 ones_row[:], start=False, stop=True)
            th = sm.tile([128, ND * T], bf16, tag="sig", bufs=1)
            nc.scalar.activation(th[:], pzz[:], AF.Tanh, scale=0.5)
            rfq = sm.tile([128, ND * T], bf16, tag="rfq", bufs=2)
            nc.vector.tensor_scalar(rfq[:], th[:], -0.5, 0.5, OP.mult, OP.add)
            mdt = sm.tile([128, ND * T], bf16, tag="mdt", bufs=2)
            nc.scalar.activation(mdt[:], rfq[:], AF.Ln)
            if DBG and j == 0:
                nc.sync.dma_start(dbg["rfq"][:], rfq[:])
                nc.sync.dma_start(dbg["mdt"][:], mdt[:])
                nc.sync.dma_start(dbg["xbc"][:], xbc[:])
            st.update(rfq=rfq, mdt=mdt, xbc=xbc)

        def stage_b4(j):
            st = state[j]
            uT, rfq, mdt, xbc = st["uT"], st["rfq"], st["mdt"], st["xbc"]
            dtu = sm.tile([128, ND * T], bf16, tag="dtu", bufs=1)
            nc.vector.tensor_tensor(out=dtu[:], in0=mdt[:], in1=uT[:], op=OP.mult)
            amega = big.tile([128, ND * N * SEG], bf16, tag="amega")
            a4 = r4(amega)
            nc.gpsimd.memset(a4[:, :, :, 0:1], 0.0)
            nc.vector.tensor_copy(
                a4[:, :, 0:1, 1:SEG],
                rfq[:].rearrange("p (d o t) -> p d o t", o=1, t=T))

            brep = big.tile([128, N * T], bf16, tag="brep")
            crep = big.tile([128, N * T], bf16, tag="crep")
            for q in range(8):
                pb = psb.tile([128, 2 * T], f32, tag="bc")
                for qq in range(2):
                    n = q * 2 + qq
                    nc.tensor.matmul(pb[:, qq * T:(qq + 1) * T],
                                     sel[0:16, n * 128:(n + 1) * 128],
                                     xbc[0:N, :], start=True, stop=True)
                nc.scalar.copy(brep[:, q * 2 * T:(q + 1) * 2 * T], pb[:])
            for q in range(8):
                pc = psb.tile([128, 2 * T], f32, tag="bc")
                for qq in range(2):
                    n = q * 2 + qq
                    nc.tensor.matmul(pc[:, qq * T:(qq + 1) * T],
                                     sel[32:48, n * 128:(n + 1) * 128],
                                     xbc[32:48, :], start=True, stop=True)
                nc.scalar.copy(crep[:, q * 2 * T:(q + 1) * 2 * T], pc[:])


            nc.vector.tensor_tensor(out=a4[:, :, 1:2, 1:SEG], in0=a4[:, :, 0:1, 1:SEG],
                                    in1=a4[:, :, 0:1, 1:SEG], op=OP.mult)
            nc.vector.tensor_tensor(out=a4[:, :, 2:4, 1:SEG], in0=a4[:, :, 0:2, 1:SEG],
                                    in1=a4[:, :, 1:2, 1:SEG].broadcast_to([128, ND, 2, T]),
                                    op=OP.mult)
            nc.vector.tensor_tensor(out=a4[:, :, 4:8, 1:SEG], in0=a4[:, :, 0:4, 1:SEG],
                                    in1=a4[:, :, 3:4, 1:SEG].broadcast_to([128, ND, 4, T]),
                                    op=OP.mult)
            nc.vector.tensor_tensor(out=a4[:, :, 8:16, 1:SEG], in0=a4[:, :, 0:8, 1:SEG],
                                    in1=a4[:, :, 7:8, 1:SEG].broadcast_to([128, ND, 8, T]),
                                    op=OP.mult)
            if DBG and j == 0:
                nc.sync.dma_start(dbg["dtu"][:], dtu[:])
                nc.sync.dma_start(dbg["brep"][:], brep[:])
                nc.sync.dma_start(dbg["crep"][:], crep[:])
                nc.sync.dma_start(dbg["amega"][:], amega[:])
            st.update(amega=amega, brep=brep, crep=crep, dtu=dtu)

        def stage_c(j):
            st = state[j]
            amega, brep, crep = st["amega"], st["brep"], st["crep"]
            dtu, uT = st["dtu"], st["uT"]
            dbus = big.tile([128, ND * N * SEG], bf16, tag="dbus")
            d4 = r4(dbus)
            if j == 0:
                nc.gpsimd.memset(d4[:, :, :, 0:1], 0.0)
            else:
                nc.gpsimd.tensor_copy(
                    d4[:, :, :, 0:1],
                    carry[:].rearrange("p (d n o) -> p d n o", n=N, o=1))
            nc.vector.tensor_tensor(
                out=d4[:, :, :, 1:SEG],
                in0=dtu[:].rearrange("p (d o t) -> p d o t", o=1, t=T)
                    .broadcast_to([128, ND, N, T]),
                in1=brep[:].rearrange("p (n t) -> p n t", t=T).unsqueeze(1)
                    .broadcast_to([128, ND, N, T]),
                op=OP.mult)
            SC = N * SEG
            h4 = r4(dbus)
            cr4 = crep[:].rearrange("p (n t) -> p n t", t=T).unsqueeze(1)
            yq = sm.tile([128, ND * T], bf16, tag="yq", bufs=3)
            py = None
            for db in range(ND):
                nc.vector.tensor_tensor_scan(
                    dbus[:, db * SC:(db + 1) * SC],
                    amega[:, db * SC:(db + 1) * SC],
                    dbus[:, db * SC:(db + 1) * SC], 0.0, OP.mult, OP.add)
                hcq = big.tile([128, N * T], bf16, tag="hcq", bufs=2)
                hc4 = hcq[:].rearrange("p (n t) -> p n t", t=T).unsqueeze(1)
                nc.vector.tensor_tensor(out=hc4, in0=h4[:, db:db + 1, :, 1:SEG],
                                        in1=cr4, op=OP.mult)
                if db % 2 == 0:
                    py = psb.tile([128, 2 * T], f32, tag="bc")
                dst = py[:, (db % 2) * T:(db % 2 + 1) * T]
                for n in range(N):
                    nc.tensor.matmul(dst, ident, hcq[:, n * T:(n + 1) * T],
                                     start=(n == 0), stop=False)
                nc.tensor.matmul(dst, dg(44 + db), uT[:, db * T:(db + 1) * T],
                                 start=False, stop=True)
                if db % 2 == 1:
                    nc.scalar.copy(yq[:, (db - 1) * T:(db + 1) * T], py[:])
            nc.gpsimd.tensor_copy(
                carry[:].rearrange("p (d n o) -> p d n o", n=N, o=1),
                h4[:, :, :, SEG - 1:SEG])
            if DBG and j == 0:
                nc.sync.dma_start(dbg["hsc"][:], dbus[:])
            if DBG and j == 0:
                nc.sync.dma_start(dbg["yq"][:], yq[:])
            st["yq"] = yq

        def stage_d(j):
            st = state.pop(j)
            yq, szT, sgT = st["yq"], st["szT"], st["sgT"]
            g1 = sm.tile([128, ND * T], bf16, tag="g1", bufs=1)
            nc.vector.tensor_tensor(out=g1[:], in0=szT[:], in1=yq[:], op=OP.mult)
            po = ps.tile([128, ND * T], f32, tag="big")
            for mb in range(ND):
                dst = po[:, mb * T:(mb + 1) * T]
                for kb in range(ND):
                    nc.tensor.matmul(dst, moT2[kb][:, mb * 128:(mb + 1) * 128],
                                     g1[:, kb * T:(kb + 1) * T],
                                     start=(kb == 0), stop=(kb == ND - 1))
            if DBG and j == 0:
                nc.sync.dma_start(dbg["g1"][:], g1[:])
            moq = sm.tile([128, ND * T], bf16, tag="moq", bufs=1)
            nc.vector.tensor_tensor(out=moq[:], in0=sgT[:], in1=po[:], op=OP.mult)
            if DBG and j == 0:
                nc.sync.dma_start(dbg["moq"][:], moq[:])
            pf = ps.tile([128, ND * T], f32, tag="big")
            for mb in range(DM // 128):
                dst = pf[:, mb * T:(mb + 1) * T]
                for kb in range(ND):
                    nc.tensor.matmul(dst, woT[kb][:, mb * 128:(mb + 1) * 128],
                                     moq[:, kb * T:(kb + 1) * T],
                                     start=(kb == 0), stop=(kb == ND - 1))
            ot = sm.tile([128, (DM // 128) * T], bf16, tag="ot", bufs=1)
            nc.scalar.copy(ot[:], pf[:, 0:(DM // 128) * T])
            otr = sm.tile([128, NTB * DM], bf16, tag="otr")
            for mb in range(DM // 128):
                for tb in range(NTB):
                    nc.sync.dma_start(
                        otr[:, tb * DM + mb * 128:tb * DM + (mb + 1) * 128],
                        ot[:, mb * T + tb * 128:mb * T + (tb + 1) * 128],
                        transpose=True)
            nc.sync.dma_start(
                part[j * T:(j + 1) * T, :].rearrange("(a p) d -> p a d", p=128),
                otr[:].rearrange("p (a d) -> p a d", d=DM))

        def fix_halo(jprev, jcur):
            cp = state[jprev]["cin"][:].rearrange("p (d c) -> p d c", c=TC9)
            cc = state[jcur]["cin"][:].rearrange("p (d c) -> p d c", c=TC9)
            nc.gpsimd.tensor_copy(cp[:, :, T + 6:T + 9], cc[:, :, 6:9])
            nc.gpsimd.tensor_copy(cc[:, :, 0:6], cp[:, :, T:T + 6])

        # ================= main loop (A, B1, B2, B3+C, D staggered) ========
        for i in range(nt + 5):
            if i < nt:
                stage_a(i)
                if i == 0:
                    nc.gpsimd.memset(
                        state[0]["cin"][:].rearrange("p (d c) -> p d c", c=TC9)
                        [:, :, 0:6], 0.0)
                else:
                    fix_halo(i - 1, i)
            if i == nt:
                nc.gpsimd.memset(
                    state[nt - 1]["cin"][:].rearrange("p (d c) -> p d c", c=TC9)
                    [:, :, T + 6:T + 9], 0.0)
            if 0 <= i - 1 < nt:
                stage_b1(i - 1)
            if 0 <= i - 2 < nt:
                stage_b2(i - 2)
            if 0 <= i - 3 < nt:
                stage_b3(i - 3)
            if 0 <= i - 4 < nt:
                stage_b4(i - 4)
                stage_c(i - 4)
            if 0 <= i - 5 < nt:
                stage_d(i - 5)

    nc.compile()
    return nc


def host_prepare(inputs, Lx=L):
    import ml_dtypes
    f32 = np.float32
    bf = ml_dtypes.bfloat16
    x = np.asarray(inputs["x"], f32)
    ln_g = np.asarray(inputs["ln_g"], f32)
    ln_b = np.asarray(inputs["ln_b"], f32)
    in_w = np.asarray(inputs["in_w"], f32)
    conv_w = np.asarray(inputs["conv_w"], f32)
    conv_b = np.asarray(inputs["conv_b"], f32)
    out_w = np.asarray(inputs["out_w"], f32)

    in_w_eff = in_w * ln_g[None, :]
    bias_vec = in_w @ ln_b

    sel = np.zeros((48, N * 128), f32)
    for n in range(N):
        sel[n, n * 128:(n + 1) * 128] = 1.0
        sel[32 + n, n * 128:(n + 1) * 128] = 1.0

    core_maps, meta = [], []
    for b in range(x.shape[0]):
        for d, p in enumerate(("f", "b")):
            m_in_w = np.asarray(inputs[p + "_in_w"], f32)
            m_conv_w = np.asarray(inputs[p + "_conv_w"], f32)
            m_conv_b = np.asarray(inputs[p + "_conv_b"], f32)
            m_xproj = np.asarray(inputs[p + "_xproj_w"], f32)
            m_dt_w = np.asarray(inputs[p + "_dt_w"], f32)
            m_dt_b = np.asarray(inputs[p + "_dt_b"], f32)
            m_D = np.asarray(inputs[p + "_D"], f32)
            m_out_w = np.asarray(inputs[p + "_out_w"], f32)

            xc = x[b] if d == 0 else x[b, ::-1]
            w7 = np.zeros((DH, CW), f32)
            if d == 0:
                w7[:, 0:4] = conv_w
            else:
                w7[:, 3:7] = conv_w[:, ::-1]
            cb_eff = conv_b + bias_vec[:DH] * conv_w.sum(axis=1)

            diags = np.zeros((128, 49 * 128), f32)
            idx = 0
            for db in range(ND):
                for k in range(CW):
                    np.fill_diagonal(diags[:, idx * 128:(idx + 1) * 128],
                                     w7[db * 128:(db + 1) * 128, k])
                    idx += 1
            for db in range(ND):
                for k in range(KC):
                    np.fill_diagonal(diags[:, idx * 128:(idx + 1) * 128],
                                     m_conv_w[db * 128:(db + 1) * 128, k])
                    idx += 1
            for db in range(ND):
                np.fill_diagonal(diags[:, idx * 128:(idx + 1) * 128],
                                 -m_D[db * 128:(db + 1) * 128])
                idx += 1
            np.fill_diagonal(diags[:, idx * 128:(idx + 1) * 128], 1.0)

            bias_rows = np.concatenate(
                [bias_vec[DH:], cb_eff, m_conv_b, m_dt_b]).reshape(1, 4 * DH)

            core_maps.append({
                "x_in": np.ascontiguousarray(xc).astype(bf),
                "w_in_T": np.ascontiguousarray(in_w_eff.T).astype(bf),
                "m_in_T": np.ascontiguousarray(m_in_w.T).astype(bf),
                "m_xproj_T": np.ascontiguousarray(m_xproj.T).astype(bf),
                "m_dt_wT": np.ascontiguousarray(m_dt_w.T).astype(bf),
                "m_out_T2": np.ascontiguousarray(-m_out_w.T).astype(bf),
                "w_out_sl_T": np.ascontiguousarray(
                    out_w[:, d * DH:(d + 1) * DH].T).astype(bf),
                "diags": diags.astype(bf),
                "bias_rows": bias_rows.astype(bf),
                "sel": sel.astype(bf),
            })
            meta.append((b, d))
    return core_maps, meta


def kernel(**inputs) -> np.ndarray:
    from concourse.bass_utils import run_bass_kernel_spmd

    if "nc" not in _CACHE:
        _CACHE["nc"] = build_program()
    nc = _CACHE["nc"]

    core_maps, meta = host_prepare(inputs)
    res = run_bass_kernel_spmd(nc, core_maps, list(range(len(core_maps))))
    x = np.asarray(inputs["x"], np.float32)
    out = np.array(x, np.float32, copy=True)
    for i, (b, d) in enumerate(meta):
        p = np.asarray(res.results[i]["part"], np.float32)
        out[b] += p if d == 0 else p[::-1]
    return out


# revision 4
# speedup vs baseline: 2261.2944x; 1.0177x over previous
"""Bass/Trainium2 kernel for nn_NeuroBiMambaBlock — engine-balanced rewrite.

Sharding: 8 cores = 4 samples x 2 directions (identical SPMD program; bwd
cores get time-flipped input + b_* weights). Within a core:
  PE   : all matmuls, depthwise convs as diag-matmul PSUM accumulation,
         B/C partition broadcasts, n-sum tree as identity-matmul PSUM
         accumulation (+ diag(-D) term), bias adds as rank-1 matmuls.
  DVE  : selective-scan (one mega tensor_tensor_scan over 4 channel
         blocks), big 2x-mode tensor_tensor streams (decay powers via
         doubling with broadcast APs, dt*u*B, h*C).
  Act  : sigmoids + Ln (2 table loads/tile), PSUM->SBUF escapes.
  Pool : overflow tensor_tensor work, boundary/carry chores.
LN stats precomputed in a prologue (Sqrt table loaded once up front).
"""

import numpy as np

B, L, DM = 4, 4096, 256
DH = 512
N = 16
KC = 4
R = 32
EPS = 1e-5
T = 256
NT = L // T              # 16 tiles
SEG = T + 1
ND = DH // 128           # 4 channel blocks
NTB = T // 128           # 2 token blocks / tile
CW = 7                   # outer wide-conv taps

_CACHE = {}
DBG = False


def build_program(Lx=L):
    import concourse.bass as bass
    import concourse.bacc as bacc
    import concourse.tile as tile
    import concourse.mybir as mybir
    from contextlib import ExitStack

    f32 = mybir.dt.float32
    bf16 = mybir.dt.bfloat16
    AF = mybir.ActivationFunctionType
    OP = mybir.AluOpType
    AX = mybir.AxisListType

    nt = Lx // T
    nblk = Lx // 128
    nc = bacc.Bacc("TRN2", target_bir_lowering=False, debug=False)

    # ---- DRAM ----
    x_in = nc.declare_dram_parameter("x_in", [Lx, DM], bf16, isOutput=False)
    w_in_T = nc.declare_dram_parameter("w_in_T", [DM, 2 * DH], bf16, isOutput=False)
    m_in_T = nc.declare_dram_parameter("m_in_T", [DH, 2 * DH], bf16, isOutput=False)
    m_xproj_T = nc.declare_dram_parameter("m_xproj_T", [DH, R + 2 * N], bf16, isOutput=False)
    m_dt_wT = nc.declare_dram_parameter("m_dt_wT", [R, DH], bf16, isOutput=False)
    m_out_T2 = nc.declare_dram_parameter("m_out_T2", [DH, DH], bf16, isOutput=False)
    w_out_sl_T = nc.declare_dram_parameter("w_out_sl_T", [DH, DM], bf16, isOutput=False)
    diags_d = nc.declare_dram_parameter("diags", [128, 49 * 128], bf16, isOutput=False)
    bias_d = nc.declare_dram_parameter("bias_rows", [1, 4 * DH], bf16, isOutput=False)
    sel_d = nc.declare_dram_parameter("sel", [48, N * 128], bf16, isOutput=False)
    part = nc.declare_dram_parameter("part", [Lx, DM], bf16, isOutput=True)
    dbg = {}
    if DBG:
        for nm, shp in (("hT", [128, 2 * T]), ("actT", [128, ND * T]),
                        ("uT", [128, ND * T]), ("sgT", [128, ND * T]),
                        ("szT", [128, ND * T]), ("rfq", [128, ND * T]),
                        ("mdt", [128, ND * T]), ("dtu", [128, ND * T]),
                        ("brep", [128, N * T]), ("crep", [128, N * T]),
                        ("amega", [128, ND * N * SEG]),
                        ("hsc", [128, ND * N * SEG]),
                        ("yq", [128, ND * T]), ("g1", [128, ND * T]),
                        ("moq", [128, ND * T]), ("xbc", [48, T])):
            dbg[nm] = nc.declare_dram_parameter("dbg_" + nm, shp, bf16, isOutput=True)

    TC9 = T + 9
    TC3 = T + 3

    with tile.TileContext(nc) as tc, ExitStack() as ctx:
        wp = ctx.enter_context(tc.tile_pool(name="w", bufs=1))
        ps = ctx.enter_context(tc.tile_pool(name="ps", bufs=3, space="PSUM"))
        psb = ctx.enter_context(tc.tile_pool(name="psb", bufs=2, space="PSUM"))
        pst = ctx.enter_context(tc.tile_pool(name="pst", bufs=2, space="PSUM"))
        sm = ctx.enter_context(tc.tile_pool(name="sm", bufs=2))
        big = ctx.enter_context(tc.tile_pool(name="big", bufs=1))
        hal = ctx.enter_context(tc.tile_pool(name="hal", bufs=2))

        # ---- weights ----
        winT = []
        for kb in range(DM // 128):
            t = wp.tile([128, 2 * DH], bf16, tag=f"winT{kb}")
            nc.sync.dma_start(t[:], w_in_T[kb * 128:(kb + 1) * 128, :])
            winT.append(t)
        minT = []
        for kb in range(ND):
            t = wp.tile([128, 2 * DH], bf16, tag=f"minT{kb}")
            nc.sync.dma_start(t[:], m_in_T[kb * 128:(kb + 1) * 128, :])
            minT.append(t)
        mxpT = []
        for kb in range(ND):
            t = wp.tile([128, R + 2 * N], bf16, tag=f"mxpT{kb}")
            nc.sync.dma_start(t[:], m_xproj_T[kb * 128:(kb + 1) * 128, :])
            mxpT.append(t)
        mdtT = wp.tile([R, DH], bf16, tag="mdtT")
        nc.sync.dma_start(mdtT[:], m_dt_wT[:])
        moT2 = []
        for kb in range(ND):
            t = wp.tile([128, DH], bf16, tag=f"moT2_{kb}")
            nc.sync.dma_start(t[:], m_out_T2[kb * 128:(kb + 1) * 128, :])
            moT2.append(t)
        woT = []
        for kb in range(ND):
            t = wp.tile([128, DM], bf16, tag=f"woT{kb}")
            nc.sync.dma_start(t[:], w_out_sl_T[kb * 128:(kb + 1) * 128, :])
            woT.append(t)
        diags = wp.tile([128, 49 * 128], bf16, tag="diags")
        nc.sync.dma_start(diags[:], diags_d[:])

        def dg(i):
            return diags[:, i * 128:(i + 1) * 128]
        ident = dg(48)
        biasr = wp.tile([1, 4 * DH], bf16, tag="biasr")
        nc.sync.dma_start(biasr[:], bias_d[:])
        sel = wp.tile([48, N * 128], bf16, tag="sel")
        nc.sync.dma_start(sel[:], sel_d[:])
        ones_row = wp.tile([1, T], bf16, tag="ones")
        nc.vector.memset(ones_row[:], 1.0)

        # ---- prologue: LN stats for all 128-token blocks ----
        mcol = wp.tile([128, nblk], f32, tag="mcol")
        vcol = wp.tile([128, nblk], f32, tag="vcol")
        rcol = wp.tile([128, nblk], f32, tag="rcol")
        for bp in range(nblk // 2):
            xb = sm.tile([128, 2 * DM], bf16, tag="otr", bufs=2)
            nc.sync.dma_start(xb[:].rearrange("p (a d) -> p a d", d=DM),
                              x_in[bp * 256:(bp + 1) * 256, :]
                              .rearrange("(a p) d -> p a d", p=128))
            sq = sm.tile([128, 2 * DM], bf16, tag="otr", bufs=2)
            nc.gpsimd.tensor_tensor(out=sq[:], in0=xb[:], in1=xb[:], op=OP.mult)
            for o in range(2):
                blk = bp * 2 + o
                s1 = sm.tile([128, 1], f32, tag="s1")
                nc.vector.tensor_reduce(s1[:], xb[:, o * DM:(o + 1) * DM],
                                        axis=AX.X, op=OP.add)
                nc.gpsimd.tensor_scalar(mcol[:, blk:blk + 1], s1[:], 1.0 / DM,
                                        None, OP.mult)
                s2 = sm.tile([128, 1], f32, tag="s2")
                nc.vector.tensor_reduce(s2[:], sq[:, o * DM:(o + 1) * DM],
                                        axis=AX.X, op=OP.add)
                nc.gpsimd.tensor_scalar(vcol[:, blk:blk + 1], s2[:], 1.0 / DM,
                                        None, OP.mult)
        msq = wp.tile([128, nblk], f32, tag="msq")
        nc.gpsimd.tensor_tensor(out=msq[:], in0=mcol[:], in1=mcol[:], op=OP.mult)
        nc.gpsimd.tensor_tensor(out=vcol[:], in0=vcol[:], in1=msq[:], op=OP.subtract)
        nc.gpsimd.tensor_scalar(vcol[:], vcol[:], EPS, None, OP.add)
        nc.vector.reciprocal(msq[:], vcol[:])
        nc.scalar.activation(rcol[:], msq[:], AF.Sqrt)

        carry = wp.tile([128, ND * N], bf16, tag="carry")

        def r4(ap_tile):
            return ap_tile[:].rearrange("p (d n c) -> p d n c", n=N, c=SEG)

        state = {}
        xin_prev = [None]

        def stage_a(i):
            """LN + transpose + outer in-proj conv half."""
            xtile = sm.tile([128, NTB * DM], bf16, tag="xt", bufs=1)
            nc.sync.dma_start(
                xtile[:].rearrange("p (a d) -> p a d", d=DM),
                x_in[i * T:(i + 1) * T, :].rearrange("(a p) d -> p a d", p=128))
            hT = sm.tile([128, (DM // 128) * T], bf16, tag="hT")
            for tb in range(NTB):
                blk = i * NTB + tb
                xln = sm.tile([128, DM], bf16, tag="xln", bufs=1)
                nc.vector.tensor_scalar(xln[:], xtile[:, tb * DM:(tb + 1) * DM],
                                        mcol[:, blk:blk + 1], rcol[:, blk:blk + 1],
                                        OP.subtract, OP.mult)
                for fb in range(DM // 128):
                    nc.sync.dma_start(
                        hT[:, fb * T + tb * 128: fb * T + (tb + 1) * 128],
                        xln[:, fb * 128:(fb + 1) * 128], transpose=True)
            pcv = ps.tile([128, ND * T], f32, tag="big")
            for mb in range(ND):
                dst = pcv[:, mb * T:(mb + 1) * T]
                for kb in range(DM // 128):
                    nc.tensor.matmul(dst, winT[kb][:, mb * 128:(mb + 1) * 128],
                                     hT[:, kb * T:(kb + 1) * T],
                                     start=(kb == 0), stop=(kb == DM // 128 - 1))
            cin = hal.tile([128, ND * TC9], bf16, tag="cin")
            nc.scalar.copy(
                cin[:].rearrange("p (d c) -> p d c", c=TC9)[:, :, 6:6 + T],
                pcv[:].rearrange("p (d t) -> p d t", t=T))
            state[i] = {"cin": cin, "hT": hT}

        def stage_b1(j):
            st = state[j]
            cin, hT = st["cin"], st["hT"]
            c3 = cin[:].rearrange("p (d c) -> p d c", c=TC9)
            pgt = ps.tile([128, ND * T], f32, tag="big")
            for mb in range(ND):
                dst = pgt[:, mb * T:(mb + 1) * T]
                nc.tensor.matmul(dst, winT[0][:, DH + mb * 128:DH + (mb + 1) * 128],
                                 hT[:, 0:T], start=True, stop=False)
                nc.tensor.matmul(dst, winT[1][:, DH + mb * 128:DH + (mb + 1) * 128],
                                 hT[:, T:2 * T], start=False, stop=False)
                nc.tensor.matmul(dst, biasr[:, mb * 128:(mb + 1) * 128],
                                 ones_row[:], start=False, stop=True)
            sgT = sm.tile([128, ND * T], bf16, tag="sgT", bufs=6)
            nc.scalar.activation(sgT[:], pgt[:], AF.Silu)
            pca = ps.tile([128, ND * T], f32, tag="big")
            for db in range(ND):
                dst = pca[:, db * T:(db + 1) * T]
                for k in range(CW):
                    nc.tensor.matmul(dst, dg(db * 7 + k), c3[:, db, 3 + k:3 + k + T],
                                     start=(k == 0), stop=False)
                nc.tensor.matmul(dst, biasr[:, DH + db * 128:DH + (db + 1) * 128],
                                 ones_row[:], start=False, stop=True)
            actT = sm.tile([128, ND * T], bf16, tag="actT")
            nc.scalar.activation(actT[:], pca[:], AF.Silu)
            if DBG and j == 0:
                nc.sync.dma_start(dbg["hT"][:], st["hT"][:])
                nc.sync.dma_start(dbg["actT"][:], actT[:])
                nc.sync.dma_start(dbg["sgT"][:], sgT[:])
            st.update(sgT=sgT, actT=actT)

        def stage_b2(j):
            st = state[j]
            actT = st["actT"]
            px = ps.tile([128, ND * T], f32, tag="big")
            pz = ps.tile([128, ND * T], f32, tag="big")
            for mb in range(ND):
                dst = px[:, mb * T:(mb + 1) * T]
                for kb in range(ND):
                    nc.tensor.matmul(dst, minT[kb][:, mb * 128:(mb + 1) * 128],
                                     actT[:, kb * T:(kb + 1) * T],
                                     start=(kb == 0), stop=(kb == ND - 1))
            for mb in range(ND):
                dst = pz[:, mb * T:(mb + 1) * T]
                for kb in range(ND):
                    nc.tensor.matmul(dst, minT[kb][:, DH + mb * 128:DH + (mb + 1) * 128],
                                     actT[:, kb * T:(kb + 1) * T],
                                     start=(kb == 0), stop=(kb == ND - 1))
            xin = hal.tile([128, ND * TC3], bf16, tag="xin")
            x3 = xin[:].rearrange("p (d c) -> p d c", c=TC3)
            nc.scalar.copy(x3[:, :, 3:3 + T],
                           px[:].rearrange("p (d t) -> p d t", t=T))
            if j == 0:
                nc.gpsimd.memset(x3[:, :, 0:3], 0.0)
            else:
                nc.gpsimd.tensor_copy(
                    x3[:, :, 0:3],
                    xin_prev[0][:].rearrange("p (d c) -> p d c", c=TC3)[:, :, T:T + 3])
            xin_prev[0] = xin
            szT = sm.tile([128, ND * T], bf16, tag="szT", bufs=5)
            nc.scalar.activation(szT[:], pz[:], AF.Silu)
            pu = ps.tile([128, ND * T], f32, tag="big")
            for db in range(ND):
                dst = pu[:, db * T:(db + 1) * T]
                for k in range(KC):
                    nc.tensor.matmul(dst, dg(28 + db * 4 + k), x3[:, db, k:k + T],
                                     start=(k == 0), stop=False)
                nc.tensor.matmul(dst, biasr[:, 2 * DH + db * 128:2 * DH + (db + 1) * 128],
                                 ones_row[:], start=False, stop=True)
            uT = sm.tile([128, ND * T], bf16, tag="uT", bufs=4)
            nc.scalar.activation(uT[:], pu[:], AF.Silu)
            if DBG and j == 0:
                nc.sync.dma_start(dbg["uT"][:], uT[:])
                nc.sync.dma_start(dbg["szT"][:], szT[:])
            st.update(szT=szT, uT=uT)

        def stage_b3(j):
            st = state[j]
            uT = st["uT"]
            pq = psb.tile([128, 2 * T], f32, tag="bc")
            for kb in range(ND):
                nc.tensor.matmul(pq[0:R, 0:T], mxpT[kb][:, 0:R],
                                 uT[:, kb * T:(kb + 1) * T],
                                 start=(kb == 0), stop=(kb == ND - 1))
            for kb in range(ND):
                nc.tensor.matmul(pq[0:N, T:2 * T], mxpT[kb][:, R:R + N],
                                 uT[:, kb * T:(kb + 1) * T],
                                 start=(kb == 0), stop=(kb == ND - 1))
            for kb in range(ND):
                nc.tensor.matmul(pq[32:48, T:2 * T], mxpT[kb][:, R + N:],
                                 uT[:, kb * T:(kb + 1) * T],
                                 start=(kb == 0), stop=(kb == ND - 1))
            xdt = sm.tile([R, T], bf16, tag="xdt", bufs=1)
            nc.scalar.copy(xdt[:], pq[0:R, 0:T])
            xbc = sm.tile([48, T], bf16, tag="xbc", bufs=2)
            nc.scalar.copy(xbc[0:N, :], pq[0:N, T:2 * T])
            nc.scalar.copy(xbc[32:48, :], pq[32:48, T:2 * T])

            pzz = ps.tile([128, ND * T], f32, tag="big")
            for db in range(ND):
                dst = pzz[:, db * T:(db + 1) * T]
                nc.tensor.matmul(dst, mdtT[:, db * 128:(db + 1) * 128],
                                 xdt[:], start=True, stop=False)
                nc.tensor.matmul(dst, biasr[:, 3 * DH + db * 128:3 * DH + (db + 1) * 128],
                                 ones_row[:], start=False, stop=True)
            th = sm.tile([128, ND * T], bf16, tag="sig", bufs=2)
            nc.scalar.activation(th[:], pzz[:], AF.Tanh, scale=0.5)
            if DBG and j == 0:
                nc.sync.dma_start(dbg["xbc"][:], xbc[:])
            st.update(th=th, xbc=xbc)

        def stage_b4(j):
            st = state[j]
            uT, th, xbc = st["uT"], st["th"], st["xbc"]
            amega = big.tile([128, ND * N * SEG], bf16, tag="amega")
            a4 = r4(amega)
            nc.gpsimd.memset(a4[:, :, :, 0:1], 0.0)
            nc.vector.tensor_scalar(
                a4[:, :, 0:1, 1:SEG],
                th[:].rearrange("p (d o t) -> p d o t", o=1, t=T),
                -0.5, 0.5, OP.mult, OP.add)
            mdt = sm.tile([128, ND * T], bf16, tag="mdt", bufs=1)
            nc.scalar.activation(
                mdt[:].rearrange("p (d o t) -> p d o t", o=1, t=T),
                a4[:, :, 0:1, 1:SEG], AF.Ln)
            if DBG and j == 0:
                nc.sync.dma_start(dbg["mdt"][:], mdt[:])
            dtu = sm.tile([128, ND * T], bf16, tag="dtu", bufs=1)
            nc.vector.tensor_tensor(out=dtu[:], in0=mdt[:], in1=uT[:], op=OP.mult)

            brep = big.tile([128, N * T], bf16, tag="brep")
            crep = big.tile([128, N * T], bf16, tag="crep")
            for q in range(8):
                pb = psb.tile([128, 2 * T], f32, tag="bc")
                for qq in range(2):
                    n = q * 2 + qq
                    nc.tensor.matmul(pb[:, qq * T:(qq + 1) * T],
                                     sel[0:16, n * 128:(n + 1) * 128],
                                     xbc[0:N, :], start=True, stop=True)
                nc.scalar.copy(brep[:, q * 2 * T:(q + 1) * 2 * T], pb[:])
            for q in range(8):
                pc = psb.tile([128, 2 * T], f32, tag="bc")
                for qq in range(2):
                    n = q * 2 + qq
                    nc.tensor.matmul(pc[:, qq * T:(qq + 1) * T],
                                     sel[32:48, n * 128:(n + 1) * 128],
                                     xbc[32:48, :], start=True, stop=True)
                nc.scalar.copy(crep[:, q * 2 * T:(q + 1) * 2 * T], pc[:])


            nc.vector.tensor_tensor(out=a4[:, :, 1:2, 1:SEG], in0=a4[:, :, 0:1, 1:SEG],
                                    in1=a4[:, :, 0:1, 1:SEG], op=OP.mult)
            nc.vector.tensor_tensor(out=a4[:, :, 2:4, 1:SEG], in0=a4[:, :, 0:2, 1:SEG],
                                    in1=a4[:, :, 1:2, 1:SEG].broadcast_to([128, ND, 2, T]),
                                    op=OP.mult)
            nc.vector.tensor_tensor(out=a4[:, :, 4:8, 1:SEG], in0=a4[:, :, 0:4, 1:SEG],
                                    in1=a4[:, :, 3:4, 1:SEG].broadcast_to([128, ND, 4, T]),
                                    op=OP.mult)
            nc.vector.tensor_tensor(out=a4[:, :, 8:16, 1:SEG], in0=a4[:, :, 0:8, 1:SEG],
                                    in1=a4[:, :, 7:8, 1:SEG].broadcast_to([128, ND, 8, T]),
                                    op=OP.mult)
            if DBG and j == 0:
                nc.sync.dma_start(dbg["dtu"][:], dtu[:])
                nc.sync.dma_start(dbg["brep"][:], brep[:])
                nc.sync.dma_start(dbg["crep"][:], crep[:])
                nc.sync.dma_start(dbg["amega"][:], amega[:])
            st.update(amega=amega, brep=brep, crep=crep, dtu=dtu)

        def stage_c(j):
            st = state[j]
            amega, brep, crep = st["amega"], st["brep"], st["crep"]
            dtu, uT = st["dtu"], st["uT"]
            dbus = big.tile([128, ND * N * SEG], bf16, tag="dbus")
            d4 = r4(dbus)
            if j == 0:
                nc.gpsimd.memset(d4[:, :, :, 0:1], 0.0)
            else:
                nc.gpsimd.tensor_copy(
                    d4[:, :, :, 0:1],
                    carry[:].rearrange("p (d n o) -> p d n o", n=N, o=1))
            nc.vector.tensor_tensor(
                out=d4[:, :, :, 1:SEG],
                in0=dtu[:].rearrange("p (d o t) -> p d o t", o=1, t=T)
                    .broadcast_to([128, ND, N, T]),
                in1=brep[:].rearrange("p (n t) -> p n t", t=T).unsqueeze(1)
                    .broadcast_to([128, ND, N, T]),
                op=OP.mult)
            SC = N * SEG
            h4 = r4(dbus)
            cr4 = crep[:].rearrange("p (n t) -> p n t", t=T).unsqueeze(1)
            yq = sm.tile([128, ND * T], bf16, tag="yq", bufs=4)
            py = None
            for db in range(ND):
                nc.vector.tensor_tensor_scan(
                    dbus[:, db * SC:(db + 1) * SC],
                    amega[:, db * SC:(db + 1) * SC],
                    dbus[:, db * SC:(db + 1) * SC], 0.0, OP.mult, OP.add)
                hcq = big.tile([128, N * T], bf16, tag="hcq", bufs=2)
                hc4 = hcq[:].rearrange("p (n t) -> p n t", t=T).unsqueeze(1)
                nc.vector.tensor_tensor(out=hc4, in0=h4[:, db:db + 1, :, 1:SEG],
                                        in1=cr4, op=OP.mult)
                if db % 2 == 0:
                    py = psb.tile([128, 2 * T], f32, tag="bc")
                dst = py[:, (db % 2) * T:(db % 2 + 1) * T]
                for n in range(N):
                    nc.tensor.matmul(dst, ident, hcq[:, n * T:(n + 1) * T],
                                     start=(n == 0), stop=False)
                nc.tensor.matmul(dst, dg(44 + db), uT[:, db * T:(db + 1) * T],
                                 start=False, stop=True)
                if db % 2 == 1:
                    nc.scalar.copy(yq[:, (db - 1) * T:(db + 1) * T], py[:])
            nc.gpsimd.tensor_copy(
                carry[:].rearrange("p (d n o) -> p d n o", n=N, o=1),
                h4[:, :, :, SEG - 1:SEG])
            if DBG and j == 0:
                nc.sync.dma_start(dbg["hsc"][:], dbus[:])
            if DBG and j == 0:
                nc.sync.dma_start(dbg["yq"][:], yq[:])
            st["yq"] = yq

        def stage_d(j):
            st = state.pop(j)
            yq, szT, sgT = st["yq"], st["szT"], st["sgT"]
            g1 = sm.tile([128, ND * T], bf16, tag="g1", bufs=1)
            nc.vector.tensor_tensor(out=g1[:], in0=szT[:], in1=yq[:], op=OP.mult)
            po = ps.tile([128, ND * T], f32, tag="big")
            for mb in range(ND):
                dst = po[:, mb * T:(mb + 1) * T]
                for kb in range(ND):
                    nc.tensor.matmul(dst, moT2[kb][:, mb * 128:(mb + 1) * 128],
                                     g1[:, kb * T:(kb + 1) * T],
                                     start=(kb == 0), stop=(kb == ND - 1))
            if DBG and j == 0:
                nc.sync.dma_start(dbg["g1"][:], g1[:])
            moq = sm.tile([128, ND * T], bf16, tag="moq", bufs=1)
            nc.vector.tensor_tensor(out=moq[:], in0=sgT[:], in1=po[:], op=OP.mult)
            if DBG and j == 0:
                nc.sync.dma_start(dbg["moq"][:], moq[:])
            pf = ps.tile([128, ND * T], f32, tag="big")
            for mb in range(DM // 128):
                dst = pf[:, mb * T:(mb + 1) * T]
                for kb in range(ND):
                    nc.tensor.matmul(dst, woT[kb][:, mb * 128:(mb + 1) * 128],
                                     moq[:, kb * T:(kb + 1) * T],
                                     start=(kb == 0), stop=(kb == ND - 1))
            ot = sm.tile([128, (DM // 128) * T], bf16, tag="ot", bufs=1)
            nc.scalar.copy(ot[:], pf[:, 0:(DM // 128) * T])
            otr = sm.tile([128, NTB * DM], bf16, tag="otr")
            for mb in range(DM // 128):
                for tb in range(NTB):
                    nc.sync.dma_start(
                        otr[:, tb * DM + mb * 128:tb * DM + (mb + 1) * 128],
                        ot[:, mb * T + tb * 128:mb * T + (tb + 1) * 128],
                        transpose=True)
            nc.sync.dma_start(
                part[j * T:(j + 1) * T, :].rearrange("(a p) d -> p a d", p=128),
                otr[:].rearrange("p (a d) -> p a d", d=DM))

        def fix_halo(jprev, jcur):
            cp = state[jprev]["cin"][:].rearrange("p (d c) -> p d c", c=TC9)
            cc = state[jcur]["cin"][:].rearrange("p (d c) -> p d c", c=TC9)
            nc.gpsimd.tensor_copy(cp[:, :, T + 6:T + 9], cc[:, :, 6:9])
            nc.gpsimd.tensor_copy(cc[:, :, 0:6], cp[:, :, T:T + 6])

        # ================= main loop (A, B1, B2, B3+C, D staggered) ========
        for i in range(nt + 6):
            if i < nt:
                stage_a(i)
                if i == 0:
                    nc.gpsimd.memset(
                        state[0]["cin"][:].rearrange("p (d c) -> p d c", c=TC9)
                        [:, :, 0:6], 0.0)
                else:
                    fix_halo(i - 1, i)
            if i == nt:
                nc.gpsimd.memset(
                    state[nt - 1]["cin"][:].rearrange("p (d c) -> p d c", c=TC9)
                    [:, :, T + 6:T + 9], 0.0)
            if 0 <= i - 1 < nt:
                stage_b1(i - 1)
            if 0 <= i - 2 < nt:
                stage_b2(i - 2)
            if 0 <= i - 3 < nt:
                stage_b3(i - 3)
            if 0 <= i - 4 < nt:
                stage_b4(i - 4)
                stage_c(i - 4)
            if 0 <= i - 6 < nt:
                stage_d(i - 6)

    nc.compile()
    return nc


def host_prepare(inputs, Lx=L):
    import ml_dtypes
    f32 = np.float32
    bf = ml_dtypes.bfloat16
    x = np.asarray(inputs["x"], f32)
    ln_g = np.asarray(inputs["ln_g"], f32)
    ln_b = np.asarray(inputs["ln_b"], f32)
    in_w = np.asarray(inputs["in_w"], f32)
    conv_w = np.asarray(inputs["conv_w"], f32)
    conv_b = np.asarray(inputs["conv_b"], f32)
    out_w = np.asarray(inputs["out_w"], f32)

    in_w_eff = in_w * ln_g[None, :]
    bias_vec = in_w @ ln_b

    sel = np.zeros((48, N * 128), f32)
    for n in range(N):
        sel[n, n * 128:(n + 1) * 128] = 1.0
        sel[32 + n, n * 128:(n + 1) * 128] = 1.0

    core_maps, meta = [], []
    for b in range(x.shape[0]):
        for d, p in enumerate(("f", "b")):
            m_in_w = np.asarray(inputs[p + "_in_w"], f32)
            m_conv_w = np.asarray(inputs[p + "_conv_w"], f32)
            m_conv_b = np.asarray(inputs[p + "_conv_b"], f32)
            m_xproj = np.asarray(inputs[p + "_xproj_w"], f32)
            m_dt_w = np.asarray(inputs[p + "_dt_w"], f32)
            m_dt_b = np.asarray(inputs[p + "_dt_b"], f32)
            m_D = np.asarray(inputs[p + "_D"], f32)
            m_out_w = np.asarray(inputs[p + "_out_w"], f32)

            xc = x[b] if d == 0 else x[b, ::-1]
            w7 = np.zeros((DH, CW), f32)
            if d == 0:
                w7[:, 0:4] = conv_w
            else:
                w7[:, 3:7] = conv_w[:, ::-1]
            cb_eff = conv_b + bias_vec[:DH] * conv_w.sum(axis=1)

            diags = np.zeros((128, 49 * 128), f32)
            idx = 0
            for db in range(ND):
                for k in range(CW):
                    np.fill_diagonal(diags[:, idx * 128:(idx + 1) * 128],
                                     w7[db * 128:(db + 1) * 128, k])
                    idx += 1
            for db in range(ND):
                for k in range(KC):
                    np.fill_diagonal(diags[:, idx * 128:(idx + 1) * 128],
                                     m_conv_w[db * 128:(db + 1) * 128, k])
                    idx += 1
            for db in range(ND):
                np.fill_diagonal(diags[:, idx * 128:(idx + 1) * 128],
                                 -m_D[db * 128:(db + 1) * 128])
                idx += 1
            np.fill_diagonal(diags[:, idx * 128:(idx + 1) * 128], 1.0)

            bias_rows = np.concatenate(
                [bias_vec[DH:], cb_eff, m_conv_b, m_dt_b]).reshape(1, 4 * DH)

            core_maps.append({
                "x_in": np.ascontiguousarray(xc).astype(bf),
                "w_in_T": np.ascontiguousarray(in_w_eff.T).astype(bf),
                "m_in_T": np.ascontiguousarray(m_in_w.T).astype(bf),
                "m_xproj_T": np.ascontiguousarray(m_xproj.T).astype(bf),
                "m_dt_wT": np.ascontiguousarray(m_dt_w.T).astype(bf),
                "m_out_T2": np.ascontiguousarray(-m_out_w.T).astype(bf),
                "w_out_sl_T": np.ascontiguousarray(
                    out_w[:, d * DH:(d + 1) * DH].T).astype(bf),
                "diags": diags.astype(bf),
                "bias_rows": bias_rows.astype(bf),
                "sel": sel.astype(bf),
            })
            meta.append((b, d))
    return core_maps, meta


def kernel(**inputs) -> np.ndarray:
    from concourse.bass_utils import run_bass_kernel_spmd

    if "nc" not in _CACHE:
        _CACHE["nc"] = build_program()
    nc = _CACHE["nc"]

    core_maps, meta = host_prepare(inputs)
    res = run_bass_kernel_spmd(nc, core_maps, list(range(len(core_maps))))
    x = np.asarray(inputs["x"], np.float32)
    out = np.array(x, np.float32, copy=True)
    for i, (b, d) in enumerate(meta):
        p = np.asarray(res.results[i]["part"], np.float32)
        out[b] += p if d == 0 else p[::-1]
    return out
